# revision 37
# baseline (speedup 1.0000x reference)
"""Causal self-attention (B=4, T=2048, C=1024, H=16) on 8 TRN2 NeuronCores.

Sharding: core c handles batch b=c//2 and head-half hh=c%2 (8 heads).
Each core computes q/k/v projections for its heads, causal attention, and a
partial output projection (row-parallel w_proj); the host sums the two
partials per batch.

v6 design (ACT=exp-only, continuous attention stream, demand-paged fillers):
- ACT runs ONLY the softmax exp. qk psum eviction is fused into DVE
  scalar_tensor_tensor ops ((psum+bias)*cos / *sin); rotate-half is a single
  DVE stream_shuffle (head dims pre-permuted host-side so rope pairs sit
  16 rows apart inside each 32-partition block); mask-multiply on gpsimd.
- The attention kt/pair/chunk loops form ONE stream: the attnV skew queue
  carries across pair and chunk boundaries, so the exp pipeline never
  drains and the scores->exp->attnV pipeline stays full end to end.
- Projection work (qk/v/out) is cut into 1-psum-bank granules that are
  pumped into the PE stream when the virtual PE clock falls behind the
  ACT clock, with require() checkpoints placed a few kt early so the
  DVE rope chain latency is hidden.
- PSUM: 2x scores tiles (4 banks) + oA/oB (2 banks) + 2x 1-bank filler
  tiles = 8 banks.
- attn@v uses a ones-column in v for softmax denominators; v bias and output
  bias fold to the host (softmax rows sum to 1).
"""

import sys

sys.path.insert(0, "/opt/trn_rl_repo")

from contextlib import ExitStack

import numpy as np

import concourse.bass as bass
import concourse.tile as tile
from concourse import bacc, mybir
from concourse.bass_utils import run_bass_kernel_spmd

F32 = mybir.dt.float32
F16 = mybir.dt.float16
AL = mybir.AluOpType
AF = mybir.ActivationFunctionType

B, T, C, H, HD = 4, 2048, 1024, 16, 64
NCORE = 8
HH = H // 2  # heads per core
NP = HH // 2  # head pairs per core
KC = C // 128  # contraction chunks
NT = T // 128  # 128-row time tiles
NQC = T // 512  # 512-query chunks
ROPE_THETA = 10000.0
SKEW = 6  # attnV trails scores by this many key tiles
SLACK = 1200.0  # pump fillers this many ns ahead of the ACT clock

# rope pairs (d, d+32) are hosted at rows (r, r+16) of each 32-row block
SWAP16 = list(range(16, 32)) + list(range(0, 16))

_CACHE = {}


def _build_module():
    nc = bacc.Bacc("TRN2", target_bir_lowering=False, debug=False)

    xT = nc.dram_tensor("xT", [C, T], F16, kind="ExternalInput")
    wq = nc.dram_tensor("wq", [C, 512], F16, kind="ExternalInput")
    wk = nc.dram_tensor("wk", [C, 512], F16, kind="ExternalInput")
    wv = nc.dram_tensor("wv", [C, 512], F16, kind="ExternalInput")
    wp = nc.dram_tensor("wp", [512, C], F16, kind="ExternalInput")
    bqk = nc.dram_tensor("bqk", [128, 2, NP], F32, kind="ExternalInput")
    cosr = nc.dram_tensor("cosr", [128, T], F16, kind="ExternalInput")
    sinp = nc.dram_tensor("sinp", [128, T], F16, kind="ExternalInput")
    mask = nc.dram_tensor("mask", [128, 2, 128], F16, kind="ExternalInput")
    y = nc.dram_tensor("y", [T, C], F16, kind="ExternalOutput")

    with tile.TileContext(nc) as tc, ExitStack() as ctx:
        consts = ctx.enter_context(tc.tile_pool(name="consts", bufs=1))
        persist = ctx.enter_context(tc.tile_pool(name="persist", bufs=1))
        xp = ctx.enter_context(tc.tile_pool(name="xp", bufs=3))
        rp = ctx.enter_context(tc.tile_pool(name="rp", bufs=3))
        ptp = ctx.enter_context(tc.tile_pool(name="ptp", bufs=SKEW + 3))
        nrm = ctx.enter_context(tc.tile_pool(name="nrm", bufs=2))
        yp = ctx.enter_context(tc.tile_pool(name="yp", bufs=4))
        scp = ctx.enter_context(tc.tile_pool(name="scp", bufs=2, space="PSUM"))
        fpl = ctx.enter_context(tc.tile_pool(name="fpl", bufs=2, space="PSUM"))
        opl = ctx.enter_context(tc.tile_pool(name="opl", bufs=1, space="PSUM"))

        # ---- constants ----
        bqk_sb = consts.tile([128, 2, NP], F32)
        wq_sb = consts.tile([128, KC, 512], F16)
        wk_sb = consts.tile([128, KC, 512], F16)
        cos_sb = consts.tile([128, T], F16)
        sin_sb = consts.tile([128, T], F16)
        mask_sb = consts.tile([128, 2, 128], F16)
        wv_sb = consts.tile([128, KC, 512], F16)
        wp_sb = consts.tile([128, 4, C], F16)

        # ---- persistent activations ----
        qT = persist.tile([128, NP, T], F16)
        kT = persist.tile([128, NP, T], F16)
        vp = persist.tile([128, NT, HH, 65], F16)
        OT = persist.tile([128, NP, T], F16)

        # ---- virtual engine clocks (ns) for filler pumping ----
        clk = {"pe": 0.0, "act": 0.0}
        xcs = {}
        norm_done = set()  # chunks whose last pair has been normalized

        def emit_xc(j, parts):
            xc = xp.tile([128, KC, 512], F16, tag="xc")
            nk = slice(j * 512, (j + 1) * 512)
            src = xT.rearrange("(kc p) t -> p kc t", p=128)[:, :, nk]
            for eng, k0, k1 in parts:
                for kc in range(k0, k1):
                    eng.dma_start(
                        out=xc[:, kc : kc + 1, :], in_=src[:, kc : kc + 1, :]
                    )
            xcs[j] = xc

        def emit_qkproj_half(j, p, which):
            # one 1-bank psum chain: q (which=0) or k (which=1) for pair p
            if j not in xcs:
                emit_xc(j, [(nc.sync, 0, 8)])
            nk = slice(j * 512, (j + 1) * 512)
            xc = xcs[j]
            wsb = wq_sb if which == 0 else wk_sb
            dstT = qT if which == 0 else kT
            ps = fpl.tile([128, 512], F32, tag="f")
            for kc in range(KC):
                nc.tensor.matmul(
                    ps[:],
                    wsb[:, kc, p * 128 : (p + 1) * 128],
                    xc[:, kc, :],
                    start=(kc == 0),
                    stop=(kc == KC - 1),
                )
            clk["pe"] += KC * 216.0
            # fused eviction: (psum + bias) * cos / * sin on DVE, then
            # rotate-half via stream_shuffle and the final add
            bap = bqk_sb[:, which, p : p + 1]
            t1 = rp.tile([128, 512], F16, tag="t1")
            s1 = rp.tile([128, 512], F16, tag="s1")
            s2 = rp.tile([128, 512], F16, tag="s2")
            nc.vector.scalar_tensor_tensor(
                t1[:], ps[:], bap, cos_sb[:, nk], AL.add, AL.mult
            )
            nc.vector.scalar_tensor_tensor(
                s1[:], ps[:], bap, sin_sb[:, nk], AL.add, AL.mult
            )
            nc.vector.stream_shuffle(s2[:], s1[:], SWAP16)
            nc.vector.tensor_add(dstT[:, p, nk], t1[:], s2[:])

        def emit_vproj_block(j, blk):
            if j not in xcs:
                emit_xc(j, [(nc.sync, 0, 8)])
            xc = xcs[j]
            ps = fpl.tile([128, 512], F32, tag="f")
            for kc in range(KC):
                nc.tensor.matmul(
                    ps[:],
                    xc[:, kc, blk * 128 : (blk + 1) * 128],
                    wv_sb[:, kc, :],
                    start=(kc == 0),
                    stop=(kc == KC - 1),
                )
            clk["pe"] += KC * 216.0
            nc.vector.tensor_copy(
                vp[:, 4 * j + blk, :, 0:64],
                ps.rearrange("p (h d) -> p h d", h=HH),
            )

        y16s = {}
        tail_flag = [False]

        def emit_oproj_half(j, tt, nn):
            t0 = j * 512 + tt * 128
            ps = fpl.tile([128, 512], F32, tag="f")
            for kc in range(4):
                nc.tensor.matmul(
                    ps[:],
                    OT[:, kc, t0 : t0 + 128],
                    wp_sb[:, kc, nn * 512 : (nn + 1) * 512],
                    start=(kc == 0),
                    stop=(kc == 3),
                )
            clk["pe"] += 4 * 216.0
            y16 = yp.tile([128, 512], F16, tag="y16")
            nc.vector.tensor_copy(y16[:], ps[:])
            eng = nc.scalar if (tt + nn) % 2 and not tail_flag[0] else nc.sync
            eng.dma_start(
                out=y[t0 : t0 + 128, nn * 512 : (nn + 1) * 512], in_=y16[:]
            )

        # ---- demand-paged filler machinery ----
        pending = {}
        order = []

        def add_filler(key, fn):
            pending[key] = fn
            order.append(key)

        tail_hold = {
            ("op", NQC - 2, 3, 0),
            ("op", NQC - 2, 3, 1),
            ("op", NQC - 2, 2, 0),
            ("op", NQC - 2, 2, 1),
        }

        def fill_ready(key):
            # oproj fillers wait until their chunk is fully normalized;
            # a few are held back to cover the epilogue's normalize latency
            if key in tail_hold and not tail_flag[0]:
                return False
            return key[0] != "op" or key[1] in norm_done

        def run_filler(key):
            fn = pending.pop(key, None)
            if fn is not None:
                fn()

        def pump(target):
            while clk["pe"] < target:
                key = next(
                    (
                        k
                        for k in order
                        if k in pending and fill_ready(k)
                    ),
                    None,
                )
                if key is None:
                    return
                run_filler(key)

        def require(*keys):
            for k in keys:
                run_filler(k)

        # ---- continuous attention stream ----
        pend = []  # queued attnV work: (j, p, kt, nkt, pt, span, co)
        oab = {}  # (j, p) -> (oA, oB), allocated at first attnV
        post = []  # deferred normalize stage-2: (due_step, closure)
        step = [0]  # global kt counter

        def flush_one():
            j, p, kt, nkt, pt, span, co = pend.pop(0)
            # the v tiles this attnV reads must be emitted by now
            require(("v", kt // 4, kt % 4))
            if kt == 0:
                oA = opl.tile([65, 512], F32, tag="oA")
                oB = opl.tile([65, 512], F32, tag="oB")
                oab[(j, p)] = (oA, oB)
            oA, oB = oab[(j, p)]
            for h, o in ((0, oA), (1, oB)):
                nc.tensor.matmul(
                    o[:, co:512],
                    vp[:, kt, p * 2 + h, :],
                    pt[:, h, 0:span],
                    start=(kt == 0),
                    stop=(kt == nkt - 1),
                )
            clk["pe"] += 2 * span * 0.4167 + 130.0
            if kt == nkt - 1:
                emit_normalize(j, p, *oab.pop((j, p)))

        def emit_normalize(j, p, oA, oB):
            # stage 1 — free the psum accumulators fast: pull out the
            # denominator rows and the unnormalized output (OT in place)
            jq = slice(j * 512, (j + 1) * 512)
            last = False
            dns, dds = [], []
            for h, o in ((0, oA), (1, oB)):
                dn = nrm.tile([1, 512], F32, tag=f"dn{h}")
                nc.vector.tensor_copy(dn[:], o[64:65, :])
                nc.vector.tensor_copy(
                    OT[h * 64 : (h + 1) * 64, p, jq], o[0:64, :]
                )
                dns.append(dn)
                if not last:
                    dd = nrm.tile([64, 8], F32, tag=f"dd{h}")
                    nc.scalar.dma_start(
                        out=dd[:],
                        in_=dn.rearrange("p (a b) -> p a b", a=64),
                    )
                    dds.append(dd)

            # stage 2 — runs a couple of kt later so the dd DMA latency
            # never blocks the DVE queue head; the final pair instead takes
            # the DMA-free (slower reciprocal) path so the epilogue never
            # waits on a DMA ring
            def stage2():
                for h in range(2):
                    if last:
                        dr = nrm.tile([1, 512], F32, tag=f"dr{h}")
                        nc.vector.reciprocal(dr[:], dns[h][:])
                    else:
                        rr = nrm.tile([64, 8], F32, tag=f"rr{h}")
                        nc.vector.reciprocal(rr[:], dds[h][:])
                        dr = nrm.tile([1, 512], F32, tag=f"dr{h}")
                        nc.scalar.dma_start(
                            out=dr.rearrange("p (a b) -> p a b", a=64),
                            in_=rr[:],
                        )
                    # full-height broadcast so the in-place multiply reads
                    # rb at the same base partition as its OT half
                    rb = nrm.tile([128, 512], F32, tag=f"rb{h}")
                    nc.gpsimd.partition_broadcast(rb[:], dr[:])
                    sl = OT[h * 64 : (h + 1) * 64, p, jq]
                    nc.vector.tensor_mul(sl, sl, rb[h * 64 : (h + 1) * 64, :])
                if p == NP - 1:
                    norm_done.add(j)

            post.append((step[0] + 2, stage2))

        def emit_attention_pair(j, p):
            nkt = 4 * (j + 1)
            require(("qk", j, p, 0), ("qk", j, p, 1))
            for kt in range(nkt):
                i = kt - 4 * j
                span = 512 if i < 0 else 512 - 128 * i
                co = 512 - span
                q0 = j * 512 + co
                # prefetch the next pair's qk projection a few kt early so
                # its rope chain latency is hidden behind attention work
                if kt == max(0, nkt - 6):
                    if p + 1 < NP:
                        require(("qk", j, p + 1, 0), ("qk", j, p + 1, 1))
                    elif j + 1 < NQC:
                        require(("qk", j + 1, 0, 0), ("qk", j + 1, 0, 1))
                # v projection of this chunk, spread over the early kts of
                # pair 0 (first consumer is the diagonal attnV, much later)
                if p == 0 and 1 <= kt <= 4:
                    require(("v", j, kt - 1))
                while post and post[0][0] <= step[0]:
                    post.pop(0)[1]()
                step[0] += 1
                if pend and pend[0][2] == 0:
                    # next flush opens a new pair: its first attnV waits for
                    # the previous pair's psum eviction — cover with fillers
                    pump(clk["pe"] + 3500.0)
                pump(clk["act"] + SLACK)
                if len(pend) > SKEW:
                    flush_one()
                sc = scp.tile([128, 2, 512], F32, tag="sc")
                for h in range(2):
                    nc.tensor.matmul(
                        sc[:, h, 0:span],
                        kT[h * 64 : (h + 1) * 64, p,
                           kt * 128 : (kt + 1) * 128],
                        qT[h * 64 : (h + 1) * 64, p, q0 : q0 + span],
                        start=True,
                        stop=True,
                        tile_position=(h * 64, 0),
                    )
                clk["pe"] += span * 0.4167 + 180.0
                pt = ptp.tile([128, 2, 512], F16, tag="pt")
                nc.scalar.activation(
                    pt[:, :, 0:span], sc[:, :, 0:span], AF.Exp
                )
                clk["act"] = max(clk["act"], clk["pe"]) + (
                    2 * span * 0.833 + 260.0
                )
                if i >= 0:
                    nc.vector.tensor_mul(
                        pt[:, :, 0:128], pt[:, :, 0:128], mask_sb[:]
                    )
                pend.append((j, p, kt, nkt, pt, span, co))

        # prologue: stripe the loads over the three DMA-capable queues so the
        # first matmul can start early.
        # rope tables/bias first (the first eviction needs them), then x and
        # the weights in first-use order; ~1KB descriptors stream fastest
        wqr = wq.rearrange("(kc p) n -> p kc n", p=128)
        nc.sync.dma_start(out=wq_sb[:, 0:1, :], in_=wqr[:, 0:1, :])
        xc0 = xp.tile([128, KC, 512], F16, tag="xc")
        xsrc = xT.rearrange("(kc p) t -> p kc t", p=128)[:, :, 0:512]
        nc.scalar.dma_start(out=xc0[:, 0:1, :], in_=xsrc[:, 0:1, :])
        nc.gpsimd.dma_start(out=xc0[:, 4:5, :], in_=xsrc[:, 4:5, :])
        xcs[0] = xc0
        nc.scalar.dma_start(out=bqk_sb[:], in_=bqk[:])
        nc.gpsimd.dma_start(out=sin_sb[:], in_=sinp[:])
        nc.scalar.dma_start(out=cos_sb[:], in_=cosr[:])
        for kc in range(1, KC, 2):
            nc.sync.dma_start(
                out=wq_sb[:, kc : kc + 2, :] if kc + 2 <= KC
                else wq_sb[:, kc:KC, :],
                in_=wqr[:, kc : kc + 2, :] if kc + 2 <= KC
                else wqr[:, kc:KC, :],
            )
        for kc in (1, 2, 3):
            nc.scalar.dma_start(out=xc0[:, kc : kc + 1, :], in_=xsrc[:, kc : kc + 1, :])
        for kc in (5, 6, 7):
            nc.gpsimd.dma_start(out=xc0[:, kc : kc + 1, :], in_=xsrc[:, kc : kc + 1, :])
        wkr = wk.rearrange("(kc p) n -> p kc n", p=128)
        for kc in range(0, KC, 4):
            nc.gpsimd.dma_start(
                out=wk_sb[:, kc : kc + 4, :], in_=wkr[:, kc : kc + 4, :]
            )
        nc.scalar.dma_start(out=mask_sb[:], in_=mask[:])
        nc.gpsimd.dma_start(out=wv_sb[:], in_=wv.rearrange("(kc p) n -> p kc n", p=128))
        nc.gpsimd.memset(vp[:, :, :, 64:65], 1.0)
        nc.sync.dma_start(out=wp_sb[:], in_=wp.rearrange("(kc r) n -> r kc n", r=128))

        for j in range(NQC):
            # register this chunk's fillers: next chunk's projections
            # interleaved with the previous chunk's output projection
            for idx in range(4):
                if j + 1 < NQC:
                    add_filler(
                        ("qk", j + 1, idx, 0),
                        lambda j=j, p=idx: emit_qkproj_half(j + 1, p, 0),
                    )
                    add_filler(
                        ("qk", j + 1, idx, 1),
                        lambda j=j, p=idx: emit_qkproj_half(j + 1, p, 1),
                    )
                if j > 0:
                    add_filler(
                        ("op", j - 1, idx, 0),
                        lambda j=j, tt=idx: emit_oproj_half(j - 1, tt, 0),
                    )
                    add_filler(
                        ("op", j - 1, idx, 1),
                        lambda j=j, tt=idx: emit_oproj_half(j - 1, tt, 1),
                    )
            if j + 1 < NQC:
                for blk in range(4):
                    add_filler(
                        ("v", j + 1, blk),
                        lambda j=j, blk=blk: emit_vproj_block(j + 1, blk),
                    )
            if j == 0:
                for idx in range(4):
                    add_filler(
                        ("qk", 0, idx, 0),
                        lambda p=idx: emit_qkproj_half(0, p, 0),
                    )
                    add_filler(
                        ("qk", 0, idx, 1),
                        lambda p=idx: emit_qkproj_half(0, p, 1),
                    )
                for blk in range(4):
                    add_filler(
                        ("v", 0, blk),
                        lambda blk=blk: emit_vproj_block(0, blk),
                    )
            for p in range(NP):
                if p == 1 and j + 1 < NQC and (j + 1) not in xcs:
                    emit_xc(j + 1, [(nc.sync, 0, 8)])
                emit_attention_pair(j, p)

        while pend:
            flush_one()
        tail_flag[0] = True
        # leftover fillers cover the final normalize chain's DMA latency
        for key in [k for k in order if k in pending and fill_ready(k)]:
            run_filler(key)
        while post:
            post.pop(0)[1]()
        for key in [k for k in order if k in pending]:
            run_filler(key)
        tail_flag[0] = False  # dd/dr drained; final stores may use both rings
        for tt in range(4):
            emit_oproj_half(NQC - 1, tt, 0)
            emit_oproj_half(NQC - 1, tt, 1)

    nc.compile()
    return nc


# row -> head-dim permutation putting rope pairs (d, d+32) at rows (r, r+16)
# inside each 32-row block, so rotate-half is a single stream_shuffle
_PI = np.concatenate(
    [np.arange(0, 16), np.arange(32, 48), np.arange(16, 32), np.arange(48, 64)]
)


def _rope_tables():
    freqs = 1.0 / (ROPE_THETA ** (np.arange(0, HD, 2, dtype=np.float32) / HD))
    ang = np.arange(T, dtype=np.float32)[:, None] * freqs[None, :]  # [T, 32]
    cos = np.cos(ang)  # [T, 32]
    sin = np.sin(ang)
    crow = cos[:, _PI % 32].T  # [64, T]
    sgn = np.where(_PI < 32, 1.0, -1.0).astype(np.float32)
    srow = (sin[:, _PI % 32] * sgn[None, :]).T
    cos_rep = np.tile(crow, (2, 1))  # [128, T]
    sin_pm = np.tile(srow, (2, 1))
    return cos_rep, sin_pm


def _f16(a):
    return np.ascontiguousarray(a).astype(np.float16)


def _prep_inputs(x, w_qkv, b_qkv, w_proj):
    cos_rep, sin_pm = _rope_tables()
    km = np.arange(128)
    mask1 = (km[:, None] <= km[None, :]).astype(np.float32)  # keep k <= q
    mask2 = np.stack([mask1, mask1], axis=1)  # [128, 2, 128]
    # per-head column permutation for the rope row layout
    colperm = np.concatenate([h * 64 + _PI for h in range(HH)])
    in_maps = []
    for c in range(NCORE):
        b, hh = c // 2, c % 2
        s = hh * 512
        m = {
            "xT": _f16(x[b].T),
            "wq": _f16(w_qkv[:, s : s + 512][:, colperm] / 8.0),
            "wk": _f16(w_qkv[:, C + s : C + s + 512][:, colperm]),
            "wv": _f16(w_qkv[:, 2 * C + s : 2 * C + s + 512]),
            "wp": _f16(w_proj[s : s + 512, :]),
            "bqk": np.stack(
                [
                    (b_qkv[s : s + 512][colperm] / 8.0).reshape(NP, 128),
                    b_qkv[C + s : C + s + 512][colperm].reshape(NP, 128),
                ]
            ).astype(np.float32).transpose(2, 0, 1),
            "cosr": _f16(cos_rep),
            "sinp": _f16(sin_pm),
            "mask": _f16(mask2),
        }
        in_maps.append(m)
    return in_maps


def _run(x, w_qkv, b_qkv, w_proj, b_proj, trace=False):
    if "nc" not in _CACHE:
        _CACHE["nc"] = _build_module()
    nc = _CACHE["nc"]
    x = np.asarray(x, np.float32)
    w_qkv = np.asarray(w_qkv, np.float32)
    b_qkv = np.asarray(b_qkv, np.float32)
    w_proj = np.asarray(w_proj, np.float32)
    b_proj = np.asarray(b_proj, np.float32)
    in_maps = _prep_inputs(x, w_qkv, b_qkv, w_proj)
    res = run_bass_kernel_spmd(nc, in_maps, core_ids=list(range(NCORE)), trace=trace)
    # host-side: sum row-parallel partials and add the folded biases
    # (attn @ (v + bv) = attn @ v + bv since softmax rows sum to 1)
    ybias = (b_qkv[2 * C :] @ w_proj + b_proj).astype(np.float32)
    out = np.empty((B, T, C), np.float32)
    for b in range(B):
        out[b] = (
            res.results[2 * b]["y"].astype(np.float32)
            + res.results[2 * b + 1]["y"].astype(np.float32)
            + ybias
        )
    return out, res


def kernel(x, w_qkv, b_qkv, w_proj, b_proj, n_heads=16):
    out, _ = _run(x, w_qkv, b_qkv, w_proj, b_proj, trace=False)
    return out


# revision 38
# speedup vs baseline: 1.0031x; 1.0031x over previous
"""Causal self-attention (B=4, T=2048, C=1024, H=16) on 8 TRN2 NeuronCores.

Sharding: core c handles batch b=c//2 and head-half hh=c%2 (8 heads).
Each core computes q/k/v projections for its heads, causal attention, and a
partial output projection (row-parallel w_proj); the host sums the two
partials per batch.

v6 design (ACT=exp-only, continuous attention stream, demand-paged fillers):
- ACT runs ONLY the softmax exp. qk psum eviction is fused into DVE
  scalar_tensor_tensor ops ((psum+bias)*cos / *sin); rotate-half is a single
  DVE stream_shuffle (head dims pre-permuted host-side so rope pairs sit
  16 rows apart inside each 32-partition block); mask-multiply on gpsimd.
- The attention kt/pair/chunk loops form ONE stream: the attnV skew queue
  carries across pair and chunk boundaries, so the exp pipeline never
  drains and the scores->exp->attnV pipeline stays full end to end.
- Projection work (qk/v/out) is cut into 1-psum-bank granules that are
  pumped into the PE stream when the virtual PE clock falls behind the
  ACT clock, with require() checkpoints placed a few kt early so the
  DVE rope chain latency is hidden.
- PSUM: 2x scores tiles (4 banks) + oA/oB (2 banks) + 2x 1-bank filler
  tiles = 8 banks.
- attn@v uses a ones-column in v for softmax denominators; v bias and output
  bias fold to the host (softmax rows sum to 1).
"""

import sys

sys.path.insert(0, "/opt/trn_rl_repo")

from contextlib import ExitStack

import numpy as np

import concourse.bass as bass
import concourse.tile as tile
from concourse import bacc, mybir
from concourse.bass_utils import run_bass_kernel_spmd

F32 = mybir.dt.float32
F16 = mybir.dt.float16
AL = mybir.AluOpType
AF = mybir.ActivationFunctionType

B, T, C, H, HD = 4, 2048, 1024, 16, 64
NCORE = 8
HH = H // 2  # heads per core
NP = HH // 2  # head pairs per core
KC = C // 128  # contraction chunks
NT = T // 128  # 128-row time tiles
NQC = T // 512  # 512-query chunks
ROPE_THETA = 10000.0
SKEW = 6  # attnV trails scores by this many key tiles
SLACK = 1200.0  # pump fillers this many ns ahead of the ACT clock

# rope pairs (d, d+32) are hosted at rows (r, r+16) of each 32-row block
SWAP16 = list(range(16, 32)) + list(range(0, 16))

_CACHE = {}


def _build_module():
    nc = bacc.Bacc("TRN2", target_bir_lowering=False, debug=False)

    xT = nc.dram_tensor("xT", [C, T], F16, kind="ExternalInput")
    wq = nc.dram_tensor("wq", [C, 512], F16, kind="ExternalInput")
    wk = nc.dram_tensor("wk", [C, 512], F16, kind="ExternalInput")
    wv = nc.dram_tensor("wv", [C, 512], F16, kind="ExternalInput")
    wp = nc.dram_tensor("wp", [512, C], F16, kind="ExternalInput")
    bqk = nc.dram_tensor("bqk", [128, 2, NP], F32, kind="ExternalInput")
    cosr = nc.dram_tensor("cosr", [128, T], F16, kind="ExternalInput")
    sinp = nc.dram_tensor("sinp", [128, T], F16, kind="ExternalInput")
    mask = nc.dram_tensor("mask", [128, 2, 128], F16, kind="ExternalInput")
    y = nc.dram_tensor("y", [T, C], F16, kind="ExternalOutput")

    with tile.TileContext(nc) as tc, ExitStack() as ctx:
        consts = ctx.enter_context(tc.tile_pool(name="consts", bufs=1))
        persist = ctx.enter_context(tc.tile_pool(name="persist", bufs=1))
        xp = ctx.enter_context(tc.tile_pool(name="xp", bufs=3))
        rp = ctx.enter_context(tc.tile_pool(name="rp", bufs=3))
        ptp = ctx.enter_context(tc.tile_pool(name="ptp", bufs=SKEW + 3))
        nrm = ctx.enter_context(tc.tile_pool(name="nrm", bufs=2))
        yp = ctx.enter_context(tc.tile_pool(name="yp", bufs=4))
        scp = ctx.enter_context(tc.tile_pool(name="scp", bufs=2, space="PSUM"))
        fpl = ctx.enter_context(tc.tile_pool(name="fpl", bufs=2, space="PSUM"))
        opl = ctx.enter_context(tc.tile_pool(name="opl", bufs=1, space="PSUM"))

        # ---- constants ----
        bqk_sb = consts.tile([128, 2, NP], F32)
        wq_sb = consts.tile([128, KC, 512], F16)
        wk_sb = consts.tile([128, KC, 512], F16)
        cos_sb = consts.tile([128, T], F16)
        sin_sb = consts.tile([128, T], F16)
        mask_sb = consts.tile([128, 2, 128], F16)
        wv_sb = consts.tile([128, KC, 512], F16)
        wp_sb = consts.tile([128, 4, C], F16)

        # ---- persistent activations ----
        qT = persist.tile([128, NP, T], F16)
        kT = persist.tile([128, NP, T], F16)
        vp = persist.tile([128, NT, HH, 65], F16)
        OT = persist.tile([128, NP, T], F16)

        # ---- virtual engine clocks (ns) for filler pumping ----
        clk = {"pe": 0.0, "act": 0.0}
        xcs = {}
        norm_done = set()  # chunks whose last pair has been normalized

        def emit_xc(j, parts):
            xc = xp.tile([128, KC, 512], F16, tag="xc")
            nk = slice(j * 512, (j + 1) * 512)
            src = xT.rearrange("(kc p) t -> p kc t", p=128)[:, :, nk]
            for eng, k0, k1 in parts:
                for kc in range(k0, k1):
                    eng.dma_start(
                        out=xc[:, kc : kc + 1, :], in_=src[:, kc : kc + 1, :]
                    )
            xcs[j] = xc

        def emit_qkproj_half(j, p, which):
            # one 1-bank psum chain: q (which=0) or k (which=1) for pair p
            if j not in xcs:
                emit_xc(j, [(nc.sync, 0, 8)])
            nk = slice(j * 512, (j + 1) * 512)
            xc = xcs[j]
            wsb = wq_sb if which == 0 else wk_sb
            dstT = qT if which == 0 else kT
            ps = fpl.tile([128, 512], F32, tag="f")
            for kc in range(KC):
                nc.tensor.matmul(
                    ps[:],
                    wsb[:, kc, p * 128 : (p + 1) * 128],
                    xc[:, kc, :],
                    start=(kc == 0),
                    stop=(kc == KC - 1),
                )
            clk["pe"] += KC * 216.0
            # fused eviction: (psum + bias) * cos / * sin on DVE, then
            # rotate-half via stream_shuffle and the final add
            bap = bqk_sb[:, which, p : p + 1]
            t1 = rp.tile([128, 512], F16, tag="t1")
            s1 = rp.tile([128, 512], F16, tag="s1")
            s2 = rp.tile([128, 512], F16, tag="s2")
            nc.vector.scalar_tensor_tensor(
                t1[:], ps[:], bap, cos_sb[:, nk], AL.add, AL.mult
            )
            nc.vector.scalar_tensor_tensor(
                s1[:], ps[:], bap, sin_sb[:, nk], AL.add, AL.mult
            )
            nc.vector.stream_shuffle(s2[:], s1[:], SWAP16)
            nc.vector.tensor_add(dstT[:, p, nk], t1[:], s2[:])

        def emit_vproj_block(j, blk):
            if j not in xcs:
                emit_xc(j, [(nc.sync, 0, 8)])
            xc = xcs[j]
            ps = fpl.tile([128, 512], F32, tag="f")
            for kc in range(KC):
                nc.tensor.matmul(
                    ps[:],
                    xc[:, kc, blk * 128 : (blk + 1) * 128],
                    wv_sb[:, kc, :],
                    start=(kc == 0),
                    stop=(kc == KC - 1),
                )
            clk["pe"] += KC * 216.0
            nc.vector.tensor_copy(
                vp[:, 4 * j + blk, :, 0:64],
                ps.rearrange("p (h d) -> p h d", h=HH),
            )

        y16s = {}
        tail_flag = [False]

        def emit_oproj_half(j, tt, nn):
            t0 = j * 512 + tt * 128
            ps = fpl.tile([128, 512], F32, tag="f")
            for kc in range(4):
                nc.tensor.matmul(
                    ps[:],
                    OT[:, kc, t0 : t0 + 128],
                    wp_sb[:, kc, nn * 512 : (nn + 1) * 512],
                    start=(kc == 0),
                    stop=(kc == 3),
                )
            clk["pe"] += 4 * 216.0
            y16 = yp.tile([128, 512], F16, tag="y16")
            nc.vector.tensor_copy(y16[:], ps[:])
            eng = nc.scalar if (tt + nn) % 2 and not tail_flag[0] else nc.sync
            eng.dma_start(
                out=y[t0 : t0 + 128, nn * 512 : (nn + 1) * 512], in_=y16[:]
            )

        # ---- demand-paged filler machinery ----
        pending = {}
        order = []

        def add_filler(key, fn):
            pending[key] = fn
            order.append(key)

        tail_hold = {
            ("op", NQC - 2, 3, 0),
            ("op", NQC - 2, 3, 1),
            ("op", NQC - 2, 2, 0),
            ("op", NQC - 2, 2, 1),
        }

        def fill_ready(key):
            # oproj fillers wait until their chunk is fully normalized;
            # a few are held back to cover the epilogue's normalize latency
            if key in tail_hold and not tail_flag[0]:
                return False
            return key[0] != "op" or key[1] in norm_done

        def run_filler(key):
            fn = pending.pop(key, None)
            if fn is not None:
                fn()

        def pump(target):
            while clk["pe"] < target:
                key = next(
                    (
                        k
                        for k in order
                        if k in pending and fill_ready(k)
                    ),
                    None,
                )
                if key is None:
                    return
                run_filler(key)

        def require(*keys):
            for k in keys:
                run_filler(k)

        # ---- continuous attention stream ----
        pend = []  # queued attnV work: (j, p, kt, nkt, pt, span, co)
        oab = {}  # (j, p) -> (oA, oB), allocated at first attnV
        post = []  # deferred normalize stage-2: (due_step, closure)
        step = [0]  # global kt counter

        def flush_one():
            j, p, kt, nkt, pt, span, co = pend.pop(0)
            # the v tiles this attnV reads must be emitted by now
            require(("v", kt // 4, kt % 4))
            if kt == 0:
                oA = opl.tile([65, 512], F32, tag="oA")
                oB = opl.tile([65, 512], F32, tag="oB")
                oab[(j, p)] = (oA, oB)
            oA, oB = oab[(j, p)]
            for h, o in ((0, oA), (1, oB)):
                nc.tensor.matmul(
                    o[:, co:512],
                    vp[:, kt, p * 2 + h, :],
                    pt[:, h, 0:span],
                    start=(kt == 0),
                    stop=(kt == nkt - 1),
                )
            clk["pe"] += 2 * span * 0.4167 + 130.0
            if kt == nkt - 1:
                emit_normalize(j, p, *oab.pop((j, p)))

        def emit_normalize(j, p, oA, oB):
            # stage 1 — free the psum accumulators fast: pull out the
            # denominator rows and the unnormalized output (OT in place)
            jq = slice(j * 512, (j + 1) * 512)
            last = False
            dns, dds = [], []
            for h, o in ((0, oA), (1, oB)):
                dn = nrm.tile([1, 512], F32, tag=f"dn{h}")
                nc.vector.tensor_copy(dn[:], o[64:65, :])
                nc.vector.tensor_copy(
                    OT[h * 64 : (h + 1) * 64, p, jq], o[0:64, :]
                )
                dns.append(dn)
                if not last:
                    dd = nrm.tile([64, 8], F32, tag=f"dd{h}")
                    nc.scalar.dma_start(
                        out=dd[:],
                        in_=dn.rearrange("p (a b) -> p a b", a=64),
                    )
                    dds.append(dd)

            # stage 2 — runs a couple of kt later so the dd DMA latency
            # never blocks the DVE queue head; the final pair instead takes
            # the DMA-free (slower reciprocal) path so the epilogue never
            # waits on a DMA ring
            def stage2():
                for h in range(2):
                    if last:
                        dr = nrm.tile([1, 512], F32, tag=f"dr{h}")
                        nc.vector.reciprocal(dr[:], dns[h][:])
                    else:
                        rr = nrm.tile([64, 8], F32, tag=f"rr{h}")
                        nc.vector.reciprocal(rr[:], dds[h][:])
                        dr = nrm.tile([1, 512], F32, tag=f"dr{h}")
                        nc.scalar.dma_start(
                            out=dr.rearrange("p (a b) -> p a b", a=64),
                            in_=rr[:],
                        )
                    # full-height broadcast so the in-place multiply reads
                    # rb at the same base partition as its OT half
                    rb = nrm.tile([128, 512], F32, tag=f"rb{h}")
                    nc.gpsimd.partition_broadcast(rb[:], dr[:])
                    sl = OT[h * 64 : (h + 1) * 64, p, jq]
                    nc.vector.tensor_mul(sl, sl, rb[h * 64 : (h + 1) * 64, :])
                if p == NP - 1:
                    norm_done.add(j)

            post.append((step[0] + 2, stage2))

        def emit_attention_pair(j, p):
            nkt = 4 * (j + 1)
            require(("qk", j, p, 0), ("qk", j, p, 1))
            for kt in range(nkt):
                i = kt - 4 * j
                span = 512 if i < 0 else 512 - 128 * i
                co = 512 - span
                q0 = j * 512 + co
                # prefetch the next pair's qk projection a few kt early so
                # its rope chain latency is hidden behind attention work
                if kt == max(0, nkt - 6):
                    if p + 1 < NP:
                        require(("qk", j, p + 1, 0), ("qk", j, p + 1, 1))
                    elif j + 1 < NQC:
                        require(("qk", j + 1, 0, 0), ("qk", j + 1, 0, 1))
                # v projection of this chunk, spread over the early kts of
                # pair 0 (first consumer is the diagonal attnV, much later)
                if p == 0 and 1 <= kt <= 4:
                    require(("v", j, kt - 1))
                while post and post[0][0] <= step[0]:
                    post.pop(0)[1]()
                step[0] += 1
                if pend and pend[0][2] == 0:
                    # next flush opens a new pair: its first attnV waits for
                    # the previous pair's psum eviction — cover with fillers
                    pump(clk["pe"] + 2500.0)
                pump(clk["act"] + SLACK)
                if len(pend) > SKEW:
                    flush_one()
                sc = scp.tile([128, 2, 512], F32, tag="sc")
                for h in range(2):
                    nc.tensor.matmul(
                        sc[:, h, 0:span],
                        kT[h * 64 : (h + 1) * 64, p,
                           kt * 128 : (kt + 1) * 128],
                        qT[h * 64 : (h + 1) * 64, p, q0 : q0 + span],
                        start=True,
                        stop=True,
                        tile_position=(h * 64, 0),
                    )
                clk["pe"] += span * 0.4167 + 180.0
                pt = ptp.tile([128, 2, 512], F16, tag="pt")
                nc.scalar.activation(
                    pt[:, :, 0:span], sc[:, :, 0:span], AF.Exp
                )
                clk["act"] = max(clk["act"], clk["pe"]) + (
                    2 * span * 0.833 + 260.0
                )
                if i >= 0:
                    nc.vector.tensor_mul(
                        pt[:, :, 0:128], pt[:, :, 0:128], mask_sb[:]
                    )
                pend.append((j, p, kt, nkt, pt, span, co))

        # prologue: stripe the loads over the three DMA-capable queues so the
        # first matmul can start early.
        # rope tables/bias first (the first eviction needs them), then x and
        # the weights in first-use order; ~1KB descriptors stream fastest
        wqr = wq.rearrange("(kc p) n -> p kc n", p=128)
        nc.sync.dma_start(out=wq_sb[:, 0:1, :], in_=wqr[:, 0:1, :])
        xc0 = xp.tile([128, KC, 512], F16, tag="xc")
        xsrc = xT.rearrange("(kc p) t -> p kc t", p=128)[:, :, 0:512]
        nc.scalar.dma_start(out=xc0[:, 0:1, :], in_=xsrc[:, 0:1, :])
        nc.gpsimd.dma_start(out=xc0[:, 4:5, :], in_=xsrc[:, 4:5, :])
        xcs[0] = xc0
        nc.scalar.dma_start(out=bqk_sb[:], in_=bqk[:])
        nc.gpsimd.dma_start(out=sin_sb[:], in_=sinp[:])
        nc.scalar.dma_start(out=cos_sb[:], in_=cosr[:])
        for kc in range(1, KC, 2):
            nc.sync.dma_start(
                out=wq_sb[:, kc : kc + 2, :] if kc + 2 <= KC
                else wq_sb[:, kc:KC, :],
                in_=wqr[:, kc : kc + 2, :] if kc + 2 <= KC
                else wqr[:, kc:KC, :],
            )
        for kc in (1, 2, 3):
            nc.scalar.dma_start(out=xc0[:, kc : kc + 1, :], in_=xsrc[:, kc : kc + 1, :])
        for kc in (5, 6, 7):
            nc.gpsimd.dma_start(out=xc0[:, kc : kc + 1, :], in_=xsrc[:, kc : kc + 1, :])
        wkr = wk.rearrange("(kc p) n -> p kc n", p=128)
        for kc in range(0, KC, 4):
            nc.gpsimd.dma_start(
                out=wk_sb[:, kc : kc + 4, :], in_=wkr[:, kc : kc + 4, :]
            )
        nc.scalar.dma_start(out=mask_sb[:], in_=mask[:])
        nc.gpsimd.dma_start(out=wv_sb[:], in_=wv.rearrange("(kc p) n -> p kc n", p=128))
        nc.gpsimd.memset(vp[:, :, :, 64:65], 1.0)
        nc.sync.dma_start(out=wp_sb[:], in_=wp.rearrange("(kc r) n -> r kc n", r=128))

        for j in range(NQC):
            # register this chunk's fillers: next chunk's projections
            # interleaved with the previous chunk's output projection
            for idx in range(4):
                if j + 1 < NQC:
                    add_filler(
                        ("qk", j + 1, idx, 0),
                        lambda j=j, p=idx: emit_qkproj_half(j + 1, p, 0),
                    )
                    add_filler(
                        ("qk", j + 1, idx, 1),
                        lambda j=j, p=idx: emit_qkproj_half(j + 1, p, 1),
                    )
                if j > 0:
                    add_filler(
                        ("op", j - 1, idx, 0),
                        lambda j=j, tt=idx: emit_oproj_half(j - 1, tt, 0),
                    )
                    add_filler(
                        ("op", j - 1, idx, 1),
                        lambda j=j, tt=idx: emit_oproj_half(j - 1, tt, 1),
                    )
            if j + 1 < NQC:
                for blk in range(4):
                    add_filler(
                        ("v", j + 1, blk),
                        lambda j=j, blk=blk: emit_vproj_block(j + 1, blk),
                    )
            if j == 0:
                for idx in range(4):
                    add_filler(
                        ("qk", 0, idx, 0),
                        lambda p=idx: emit_qkproj_half(0, p, 0),
                    )
                    add_filler(
                        ("qk", 0, idx, 1),
                        lambda p=idx: emit_qkproj_half(0, p, 1),
                    )
                for blk in range(4):
                    add_filler(
                        ("v", 0, blk),
                        lambda blk=blk: emit_vproj_block(0, blk),
                    )
            for p in range(NP):
                if p == 1 and j + 1 < NQC and (j + 1) not in xcs:
                    emit_xc(j + 1, [(nc.sync, 0, 8)])
                emit_attention_pair(j, p)

        while pend:
            flush_one()
        tail_flag[0] = True
        # leftover fillers cover the final normalize chain's DMA latency
        for key in [k for k in order if k in pending and fill_ready(k)]:
            run_filler(key)
        while post:
            post.pop(0)[1]()
        for key in [k for k in order if k in pending]:
            run_filler(key)
        tail_flag[0] = False  # dd/dr drained; final stores may use both rings
        for tt in range(4):
            emit_oproj_half(NQC - 1, tt, 0)
            emit_oproj_half(NQC - 1, tt, 1)

    nc.compile()
    return nc


# row -> head-dim permutation putting rope pairs (d, d+32) at rows (r, r+16)
# inside each 32-row block, so rotate-half is a single stream_shuffle
_PI = np.concatenate(
    [np.arange(0, 16), np.arange(32, 48), np.arange(16, 32), np.arange(48, 64)]
)


def _rope_tables():
    freqs = 1.0 / (ROPE_THETA ** (np.arange(0, HD, 2, dtype=np.float32) / HD))
    ang = np.arange(T, dtype=np.float32)[:, None] * freqs[None, :]  # [T, 32]
    cos = np.cos(ang)  # [T, 32]
    sin = np.sin(ang)
    crow = cos[:, _PI % 32].T  # [64, T]
    sgn = np.where(_PI < 32, 1.0, -1.0).astype(np.float32)
    srow = (sin[:, _PI % 32] * sgn[None, :]).T
    cos_rep = np.tile(crow, (2, 1))  # [128, T]
    sin_pm = np.tile(srow, (2, 1))
    return cos_rep, sin_pm


def _f16(a):
    return np.ascontiguousarray(a).astype(np.float16)


def _prep_inputs(x, w_qkv, b_qkv, w_proj):
    cos_rep, sin_pm = _rope_tables()
    km = np.arange(128)
    mask1 = (km[:, None] <= km[None, :]).astype(np.float32)  # keep k <= q
    mask2 = np.stack([mask1, mask1], axis=1)  # [128, 2, 128]
    # per-head column permutation for the rope row layout
    colperm = np.concatenate([h * 64 + _PI for h in range(HH)])
    in_maps = []
    for c in range(NCORE):
        b, hh = c // 2, c % 2
        s = hh * 512
        m = {
            "xT": _f16(x[b].T),
            "wq": _f16(w_qkv[:, s : s + 512][:, colperm] / 8.0),
            "wk": _f16(w_qkv[:, C + s : C + s + 512][:, colperm]),
            "wv": _f16(w_qkv[:, 2 * C + s : 2 * C + s + 512]),
            "wp": _f16(w_proj[s : s + 512, :]),
            "bqk": np.stack(
                [
                    (b_qkv[s : s + 512][colperm] / 8.0).reshape(NP, 128),
                    b_qkv[C + s : C + s + 512][colperm].reshape(NP, 128),
                ]
            ).astype(np.float32).transpose(2, 0, 1),
            "cosr": _f16(cos_rep),
            "sinp": _f16(sin_pm),
            "mask": _f16(mask2),
        }
        in_maps.append(m)
    return in_maps


def _run(x, w_qkv, b_qkv, w_proj, b_proj, trace=False):
    if "nc" not in _CACHE:
        _CACHE["nc"] = _build_module()
    nc = _CACHE["nc"]
    x = np.asarray(x, np.float32)
    w_qkv = np.asarray(w_qkv, np.float32)
    b_qkv = np.asarray(b_qkv, np.float32)
    w_proj = np.asarray(w_proj, np.float32)
    b_proj = np.asarray(b_proj, np.float32)
    in_maps = _prep_inputs(x, w_qkv, b_qkv, w_proj)
    res = run_bass_kernel_spmd(nc, in_maps, core_ids=list(range(NCORE)), trace=trace)
    # host-side: sum row-parallel partials and add the folded biases
    # (attn @ (v + bv) = attn @ v + bv since softmax rows sum to 1)
    ybias = (b_qkv[2 * C :] @ w_proj + b_proj).astype(np.float32)
    out = np.empty((B, T, C), np.float32)
    for b in range(B):
        out[b] = (
            res.results[2 * b]["y"].astype(np.float32)
            + res.results[2 * b + 1]["y"].astype(np.float32)
            + ybias
        )
    return out, res


def kernel(x, w_qkv, b_qkv, w_proj, b_proj, n_heads=16):
    out, _ = _run(x, w_qkv, b_qkv, w_proj, b_proj, trace=False)
    return out


# revision 39
# speedup vs baseline: 1.0223x; 1.0192x over previous
"""Causal self-attention (B=4, T=2048, C=1024, H=16) on 8 TRN2 NeuronCores.

Sharding: core c handles batch b=c//2 and head-half hh=c%2 (8 heads).
Each core computes q/k/v projections for its heads, causal attention, and a
partial output projection (row-parallel w_proj); the host sums the two
partials per batch.

v6 design (ACT=exp-only, continuous attention stream, demand-paged fillers):
- ACT runs ONLY the softmax exp. qk psum eviction is fused into DVE
  scalar_tensor_tensor ops ((psum+bias)*cos / *sin); rotate-half is a single
  DVE stream_shuffle (head dims pre-permuted host-side so rope pairs sit
  16 rows apart inside each 32-partition block); mask-multiply on gpsimd.
- The attention kt/pair/chunk loops form ONE stream: the attnV skew queue
  carries across pair and chunk boundaries, so the exp pipeline never
  drains and the scores->exp->attnV pipeline stays full end to end.
- Projection work (qk/v/out) is cut into 1-psum-bank granules that are
  pumped into the PE stream when the virtual PE clock falls behind the
  ACT clock, with require() checkpoints placed a few kt early so the
  DVE rope chain latency is hidden.
- PSUM: 2x scores tiles (4 banks) + oA/oB (2 banks) + 2x 1-bank filler
  tiles = 8 banks.
- attn@v uses a ones-column in v for softmax denominators; v bias and output
  bias fold to the host (softmax rows sum to 1).
"""

import sys

sys.path.insert(0, "/opt/trn_rl_repo")

from contextlib import ExitStack

import numpy as np

import concourse.bass as bass
import concourse.tile as tile
from concourse import bacc, mybir
from concourse.bass_utils import run_bass_kernel_spmd

F32 = mybir.dt.float32
F16 = mybir.dt.float16
AL = mybir.AluOpType
AF = mybir.ActivationFunctionType

B, T, C, H, HD = 4, 2048, 1024, 16, 64
NCORE = 8
HH = H // 2  # heads per core
NP = HH // 2  # head pairs per core
KC = C // 128  # contraction chunks
NT = T // 128  # 128-row time tiles
NQC = T // 512  # 512-query chunks
ROPE_THETA = 10000.0
SKEW = 6  # attnV trails scores by this many key tiles
SLACK = 1200.0  # pump fillers this many ns ahead of the ACT clock

# rope pairs (d, d+32) are hosted at rows (r, r+16) of each 32-row block
SWAP16 = list(range(16, 32)) + list(range(0, 16))

_CACHE = {}


def _build_module():
    nc = bacc.Bacc("TRN2", target_bir_lowering=False, debug=False)

    xT = nc.dram_tensor("xT", [C, T], F16, kind="ExternalInput")
    wq = nc.dram_tensor("wq", [C, 512], F16, kind="ExternalInput")
    wk = nc.dram_tensor("wk", [C, 512], F16, kind="ExternalInput")
    wv = nc.dram_tensor("wv", [C, 512], F16, kind="ExternalInput")
    wp = nc.dram_tensor("wp", [512, C], F16, kind="ExternalInput")
    bqk = nc.dram_tensor("bqk", [128, 2, NP], F32, kind="ExternalInput")
    cosr = nc.dram_tensor("cosr", [128, T], F16, kind="ExternalInput")
    sinp = nc.dram_tensor("sinp", [128, T], F16, kind="ExternalInput")
    mask = nc.dram_tensor("mask", [128, 2, 128], F16, kind="ExternalInput")
    y = nc.dram_tensor("y", [T, C], F16, kind="ExternalOutput")

    with tile.TileContext(nc) as tc, ExitStack() as ctx:
        consts = ctx.enter_context(tc.tile_pool(name="consts", bufs=1))
        persist = ctx.enter_context(tc.tile_pool(name="persist", bufs=1))
        xp = ctx.enter_context(tc.tile_pool(name="xp", bufs=3))
        rp = ctx.enter_context(tc.tile_pool(name="rp", bufs=3))
        ptp = ctx.enter_context(tc.tile_pool(name="ptp", bufs=SKEW + 3))
        nrm = ctx.enter_context(tc.tile_pool(name="nrm", bufs=2))
        yp = ctx.enter_context(tc.tile_pool(name="yp", bufs=4))
        scp = ctx.enter_context(tc.tile_pool(name="scp", bufs=2, space="PSUM"))
        fpl = ctx.enter_context(tc.tile_pool(name="fpl", bufs=2, space="PSUM"))
        opl = ctx.enter_context(tc.tile_pool(name="opl", bufs=1, space="PSUM"))

        # ---- constants ----
        bqk_sb = consts.tile([128, 2, NP], F32)
        wq_sb = consts.tile([128, KC, 512], F16)
        wk_sb = consts.tile([128, KC, 512], F16)
        cos_sb = consts.tile([128, T], F16)
        sin_sb = consts.tile([128, T], F16)
        mask_sb = consts.tile([128, 2, 128], F16)
        wv_sb = consts.tile([128, KC, 512], F16)
        wp_sb = consts.tile([128, 4, C], F16)

        # ---- persistent activations ----
        qT = persist.tile([128, NP, T], F16)
        kT = persist.tile([128, NP, T], F16)
        vp = persist.tile([128, NT, HH, 65], F16)
        OT = persist.tile([128, NP, T], F16)

        # ---- virtual engine clocks (ns) for filler pumping ----
        clk = {"pe": 0.0, "act": 0.0}
        xcs = {}
        norm_done = set()  # chunks whose last pair has been normalized

        def emit_xc(j, parts):
            xc = xp.tile([128, KC, 512], F16, tag="xc")
            nk = slice(j * 512, (j + 1) * 512)
            src = xT.rearrange("(kc p) t -> p kc t", p=128)[:, :, nk]
            for eng, k0, k1 in parts:
                for kc in range(k0, k1):
                    eng.dma_start(
                        out=xc[:, kc : kc + 1, :], in_=src[:, kc : kc + 1, :]
                    )
            xcs[j] = xc

        def emit_qkproj_half(j, p, which):
            # one 1-bank psum chain: q (which=0) or k (which=1) for pair p
            if j not in xcs:
                emit_xc(j, [(nc.sync, 0, 8)])
            nk = slice(j * 512, (j + 1) * 512)
            xc = xcs[j]
            wsb = wq_sb if which == 0 else wk_sb
            dstT = qT if which == 0 else kT
            ps = fpl.tile([128, 512], F32, tag="f")
            for kc in range(KC):
                nc.tensor.matmul(
                    ps[:],
                    wsb[:, kc, p * 128 : (p + 1) * 128],
                    xc[:, kc, :],
                    start=(kc == 0),
                    stop=(kc == KC - 1),
                )
            clk["pe"] += KC * 216.0
            # fused eviction: (psum + bias) * cos / * sin on DVE, then
            # rotate-half via stream_shuffle and the final add
            bap = bqk_sb[:, which, p : p + 1]
            t1 = rp.tile([128, 512], F16, tag="t1")
            s1 = rp.tile([128, 512], F16, tag="s1")
            s2 = rp.tile([128, 512], F16, tag="s2")
            nc.vector.scalar_tensor_tensor(
                t1[:], ps[:], bap, cos_sb[:, nk], AL.add, AL.mult
            )
            nc.vector.scalar_tensor_tensor(
                s1[:], ps[:], bap, sin_sb[:, nk], AL.add, AL.mult
            )
            nc.vector.stream_shuffle(s2[:], s1[:], SWAP16)
            nc.vector.tensor_add(dstT[:, p, nk], t1[:], s2[:])

        def emit_vproj_block(j, blk):
            if j not in xcs:
                emit_xc(j, [(nc.sync, 0, 8)])
            xc = xcs[j]
            ps = fpl.tile([128, 512], F32, tag="f")
            for kc in range(KC):
                nc.tensor.matmul(
                    ps[:],
                    xc[:, kc, blk * 128 : (blk + 1) * 128],
                    wv_sb[:, kc, :],
                    start=(kc == 0),
                    stop=(kc == KC - 1),
                )
            clk["pe"] += KC * 216.0
            nc.vector.tensor_copy(
                vp[:, 4 * j + blk, :, 0:64],
                ps.rearrange("p (h d) -> p h d", h=HH),
            )

        y16s = {}
        tail_flag = [False]

        def emit_oproj_half(j, tt, nn):
            t0 = j * 512 + tt * 128
            ps = fpl.tile([128, 512], F32, tag="f")
            for kc in range(4):
                nc.tensor.matmul(
                    ps[:],
                    OT[:, kc, t0 : t0 + 128],
                    wp_sb[:, kc, nn * 512 : (nn + 1) * 512],
                    start=(kc == 0),
                    stop=(kc == 3),
                )
            clk["pe"] += 4 * 216.0
            y16 = yp.tile([128, 512], F16, tag="y16")
            nc.vector.tensor_copy(y16[:], ps[:])
            eng = nc.scalar if (tt + nn) % 2 and not tail_flag[0] else nc.sync
            eng.dma_start(
                out=y[t0 : t0 + 128, nn * 512 : (nn + 1) * 512], in_=y16[:]
            )

        # ---- demand-paged filler machinery ----
        pending = {}
        order = []

        def add_filler(key, fn):
            pending[key] = fn
            order.append(key)

        tail_hold = {
            ("op", NQC - 2, 3, 0),
            ("op", NQC - 2, 3, 1),
            ("op", NQC - 2, 2, 0),
            ("op", NQC - 2, 2, 1),
        }

        def fill_ready(key):
            # oproj fillers wait until their chunk is fully normalized;
            # a few are held back to cover the epilogue's normalize latency
            if key in tail_hold and not tail_flag[0]:
                return False
            return key[0] != "op" or key[1] in norm_done

        def run_filler(key):
            fn = pending.pop(key, None)
            if fn is not None:
                fn()

        def pump(target):
            while clk["pe"] < target:
                key = next(
                    (
                        k
                        for k in order
                        if k in pending and fill_ready(k)
                    ),
                    None,
                )
                if key is None:
                    return
                run_filler(key)

        def require(*keys):
            for k in keys:
                run_filler(k)

        # ---- continuous attention stream ----
        pend = []  # queued attnV work: (j, p, kt, nkt, pt, span, co)
        oab = {}  # (j, p) -> (oA, oB), allocated at first attnV
        post = []  # deferred normalize stage-2: (due_step, closure)
        step = [0]  # global kt counter

        def flush_one():
            j, p, kt, nkt, pt, span, co = pend.pop(0)
            # the v tiles this attnV reads must be emitted by now
            require(("v", kt // 4, kt % 4))
            if kt == 0:
                oA = opl.tile([65, 512], F32, tag="oA")
                oB = opl.tile([65, 512], F32, tag="oB")
                oab[(j, p)] = (oA, oB)
            oA, oB = oab[(j, p)]
            for h, o in ((0, oA), (1, oB)):
                nc.tensor.matmul(
                    o[:, co:512],
                    vp[:, kt, p * 2 + h, :],
                    pt[:, h, 0:span],
                    start=(kt == 0),
                    stop=(kt == nkt - 1),
                )
            clk["pe"] += 2 * span * 0.4167 + 130.0
            if kt == nkt - 1:
                emit_normalize(j, p, *oab.pop((j, p)))

        def emit_normalize(j, p, oA, oB):
            # stage 1 — free the psum accumulators fast: pull out the
            # denominator rows and the unnormalized output (OT in place)
            jq = slice(j * 512, (j + 1) * 512)
            last = False
            dns, dds = [], []
            for h, o in ((0, oA), (1, oB)):
                dn = nrm.tile([1, 512], F32, tag=f"dn{h}")
                nc.vector.tensor_copy(dn[:], o[64:65, :])
                nc.vector.tensor_copy(
                    OT[h * 64 : (h + 1) * 64, p, jq], o[0:64, :]
                )
                dns.append(dn)
                if not last:
                    dd = nrm.tile([64, 8], F32, tag=f"dd{h}")
                    nc.scalar.dma_start(
                        out=dd[:],
                        in_=dn.rearrange("p (a b) -> p a b", a=64),
                    )
                    dds.append(dd)

            # stage 2 — runs a couple of kt later so the dd DMA latency
            # never blocks the DVE queue head; the final pair instead takes
            # the DMA-free (slower reciprocal) path so the epilogue never
            # waits on a DMA ring
            def stage2():
                for h in range(2):
                    if last:
                        dr = nrm.tile([1, 512], F32, tag=f"dr{h}")
                        nc.vector.reciprocal(dr[:], dns[h][:])
                    else:
                        rr = nrm.tile([64, 8], F32, tag=f"rr{h}")
                        nc.vector.reciprocal(rr[:], dds[h][:])
                        dr = nrm.tile([1, 512], F32, tag=f"dr{h}")
                        nc.scalar.dma_start(
                            out=dr.rearrange("p (a b) -> p a b", a=64),
                            in_=rr[:],
                        )
                    # full-height broadcast so the in-place multiply reads
                    # rb at the same base partition as its OT half
                    rb = nrm.tile([128, 512], F32, tag=f"rb{h}")
                    nc.gpsimd.partition_broadcast(rb[:], dr[:])
                    sl = OT[h * 64 : (h + 1) * 64, p, jq]
                    nc.vector.tensor_mul(sl, sl, rb[h * 64 : (h + 1) * 64, :])
                if p == NP - 1:
                    norm_done.add(j)

            post.append((step[0] + 2, stage2))

        def emit_attention_pair(j, p):
            nkt = 4 * (j + 1)
            require(("qk", j, p, 0), ("qk", j, p, 1))
            for kt in range(nkt):
                i = kt - 4 * j
                span = 512 if i < 0 else 512 - 128 * i
                co = 512 - span
                q0 = j * 512 + co
                # prefetch the next pair's qk projection a few kt early so
                # its rope chain latency is hidden behind attention work
                if kt == max(0, nkt - 6):
                    if p + 1 < NP:
                        require(("qk", j, p + 1, 0), ("qk", j, p + 1, 1))
                    elif j + 1 < NQC:
                        require(("qk", j + 1, 0, 0), ("qk", j + 1, 0, 1))
                # v projection of this chunk, spread over the early kts of
                # pair 0 (first consumer is the diagonal attnV, much later)
                if p == 0 and 1 <= kt <= 4:
                    require(("v", j, kt - 1))
                while post and post[0][0] <= step[0]:
                    post.pop(0)[1]()
                step[0] += 1
                if pend and pend[0][2] == 0:
                    # next flush opens a new pair: its first attnV waits for
                    # the previous pair's psum eviction — cover with fillers
                    pump(clk["pe"] + 2500.0)
                pump(clk["act"] + SLACK)
                if len(pend) > SKEW:
                    flush_one()
                sc = scp.tile([128, 2, 512], F32, tag="sc")
                for h in range(2):
                    nc.tensor.matmul(
                        sc[:, h, 0:span],
                        kT[h * 64 : (h + 1) * 64, p,
                           kt * 128 : (kt + 1) * 128],
                        qT[h * 64 : (h + 1) * 64, p, q0 : q0 + span],
                        start=True,
                        stop=True,
                        tile_position=(h * 64, 0),
                    )
                clk["pe"] += span * 0.4167 + 180.0
                pt = ptp.tile([128, 2, 512], F16, tag="pt")
                nc.scalar.activation(
                    pt[:, :, 0:span], sc[:, :, 0:span], AF.Exp
                )
                clk["act"] = max(clk["act"], clk["pe"]) + (
                    2 * span * 0.833 + 260.0
                )
                if i >= 0:
                    nc.vector.tensor_mul(
                        pt[:, :, 0:128], pt[:, :, 0:128], mask_sb[:]
                    )
                pend.append((j, p, kt, nkt, pt, span, co))

        # prologue: stripe the loads over the three DMA-capable queues so the
        # first matmul can start early.
        # rope tables/bias first (the first eviction needs them), then x and
        # the weights in first-use order; ~1KB descriptors stream fastest
        wqr = wq.rearrange("(kc p) n -> p kc n", p=128)
        nc.sync.dma_start(out=wq_sb[:, 0:1, :], in_=wqr[:, 0:1, :])
        xc0 = xp.tile([128, KC, 512], F16, tag="xc")
        xsrc = xT.rearrange("(kc p) t -> p kc t", p=128)[:, :, 0:512]
        nc.scalar.dma_start(out=xc0[:, 0:1, :], in_=xsrc[:, 0:1, :])
        nc.gpsimd.dma_start(out=xc0[:, 4:5, :], in_=xsrc[:, 4:5, :])
        xcs[0] = xc0
        for kc in (1, 2, 3):
            nc.scalar.dma_start(out=xc0[:, kc : kc + 1, :], in_=xsrc[:, kc : kc + 1, :])
        for kc in (5, 6, 7):
            nc.gpsimd.dma_start(out=xc0[:, kc : kc + 1, :], in_=xsrc[:, kc : kc + 1, :])
        for kc in range(1, KC, 2):
            nc.sync.dma_start(
                out=wq_sb[:, kc : kc + 2, :] if kc + 2 <= KC
                else wq_sb[:, kc:KC, :],
                in_=wqr[:, kc : kc + 2, :] if kc + 2 <= KC
                else wqr[:, kc:KC, :],
            )
        nc.scalar.dma_start(out=bqk_sb[:], in_=bqk[:])
        # only the first chunk of the rope tables is needed for chunk-0
        nc.scalar.dma_start(out=cos_sb[:, 0:512], in_=cosr[:, 0:512])
        nc.gpsimd.dma_start(out=sin_sb[:, 0:512], in_=sinp[:, 0:512])
        nc.scalar.dma_start(out=cos_sb[:, 512:T], in_=cosr[:, 512:T])
        nc.gpsimd.dma_start(out=sin_sb[:, 512:T], in_=sinp[:, 512:T])
        wkr = wk.rearrange("(kc p) n -> p kc n", p=128)
        for kc in range(0, KC, 4):
            nc.gpsimd.dma_start(
                out=wk_sb[:, kc : kc + 4, :], in_=wkr[:, kc : kc + 4, :]
            )
        nc.scalar.dma_start(out=mask_sb[:], in_=mask[:])
        nc.gpsimd.dma_start(out=wv_sb[:], in_=wv.rearrange("(kc p) n -> p kc n", p=128))
        nc.gpsimd.memset(vp[:, :, :, 64:65], 1.0)
        nc.sync.dma_start(out=wp_sb[:], in_=wp.rearrange("(kc r) n -> r kc n", r=128))

        for j in range(NQC):
            # register this chunk's fillers: next chunk's projections
            # interleaved with the previous chunk's output projection
            for idx in range(4):
                if j + 1 < NQC:
                    add_filler(
                        ("qk", j + 1, idx, 0),
                        lambda j=j, p=idx: emit_qkproj_half(j + 1, p, 0),
                    )
                    add_filler(
                        ("qk", j + 1, idx, 1),
                        lambda j=j, p=idx: emit_qkproj_half(j + 1, p, 1),
                    )
                if j > 0:
                    add_filler(
                        ("op", j - 1, idx, 0),
                        lambda j=j, tt=idx: emit_oproj_half(j - 1, tt, 0),
                    )
                    add_filler(
                        ("op", j - 1, idx, 1),
                        lambda j=j, tt=idx: emit_oproj_half(j - 1, tt, 1),
                    )
            if j + 1 < NQC:
                for blk in range(4):
                    add_filler(
                        ("v", j + 1, blk),
                        lambda j=j, blk=blk: emit_vproj_block(j + 1, blk),
                    )
            if j == 0:
                for idx in range(4):
                    add_filler(
                        ("qk", 0, idx, 0),
                        lambda p=idx: emit_qkproj_half(0, p, 0),
                    )
                    add_filler(
                        ("qk", 0, idx, 1),
                        lambda p=idx: emit_qkproj_half(0, p, 1),
                    )
                for blk in range(4):
                    add_filler(
                        ("v", 0, blk),
                        lambda blk=blk: emit_vproj_block(0, blk),
                    )
            for p in range(NP):
                if p == 1 and j + 1 < NQC and (j + 1) not in xcs:
                    emit_xc(j + 1, [(nc.sync, 0, 8)])
                emit_attention_pair(j, p)

        while pend:
            flush_one()
        tail_flag[0] = True
        # leftover fillers cover the final normalize chain's DMA latency
        for key in [k for k in order if k in pending and fill_ready(k)]:
            run_filler(key)
        while post:
            post.pop(0)[1]()
        for key in [k for k in order if k in pending]:
            run_filler(key)
        tail_flag[0] = False  # dd/dr drained; final stores may use both rings
        for tt in range(4):
            emit_oproj_half(NQC - 1, tt, 0)
            emit_oproj_half(NQC - 1, tt, 1)

    nc.compile()
    return nc


# row -> head-dim permutation putting rope pairs (d, d+32) at rows (r, r+16)
# inside each 32-row block, so rotate-half is a single stream_shuffle
_PI = np.concatenate(
    [np.arange(0, 16), np.arange(32, 48), np.arange(16, 32), np.arange(48, 64)]
)


def _rope_tables():
    freqs = 1.0 / (ROPE_THETA ** (np.arange(0, HD, 2, dtype=np.float32) / HD))
    ang = np.arange(T, dtype=np.float32)[:, None] * freqs[None, :]  # [T, 32]
    cos = np.cos(ang)  # [T, 32]
    sin = np.sin(ang)
    crow = cos[:, _PI % 32].T  # [64, T]
    sgn = np.where(_PI < 32, 1.0, -1.0).astype(np.float32)
    srow = (sin[:, _PI % 32] * sgn[None, :]).T
    cos_rep = np.tile(crow, (2, 1))  # [128, T]
    sin_pm = np.tile(srow, (2, 1))
    return cos_rep, sin_pm


def _f16(a):
    return np.ascontiguousarray(a).astype(np.float16)


def _prep_inputs(x, w_qkv, b_qkv, w_proj):
    cos_rep, sin_pm = _rope_tables()
    km = np.arange(128)
    mask1 = (km[:, None] <= km[None, :]).astype(np.float32)  # keep k <= q
    mask2 = np.stack([mask1, mask1], axis=1)  # [128, 2, 128]
    # per-head column permutation for the rope row layout
    colperm = np.concatenate([h * 64 + _PI for h in range(HH)])
    in_maps = []
    for c in range(NCORE):
        b, hh = c // 2, c % 2
        s = hh * 512
        m = {
            "xT": _f16(x[b].T),
            "wq": _f16(w_qkv[:, s : s + 512][:, colperm] / 8.0),
            "wk": _f16(w_qkv[:, C + s : C + s + 512][:, colperm]),
            "wv": _f16(w_qkv[:, 2 * C + s : 2 * C + s + 512]),
            "wp": _f16(w_proj[s : s + 512, :]),
            "bqk": np.stack(
                [
                    (b_qkv[s : s + 512][colperm] / 8.0).reshape(NP, 128),
                    b_qkv[C + s : C + s + 512][colperm].reshape(NP, 128),
                ]
            ).astype(np.float32).transpose(2, 0, 1),
            "cosr": _f16(cos_rep),
            "sinp": _f16(sin_pm),
            "mask": _f16(mask2),
        }
        in_maps.append(m)
    return in_maps


def _run(x, w_qkv, b_qkv, w_proj, b_proj, trace=False):
    if "nc" not in _CACHE:
        _CACHE["nc"] = _build_module()
    nc = _CACHE["nc"]
    x = np.asarray(x, np.float32)
    w_qkv = np.asarray(w_qkv, np.float32)
    b_qkv = np.asarray(b_qkv, np.float32)
    w_proj = np.asarray(w_proj, np.float32)
    b_proj = np.asarray(b_proj, np.float32)
    in_maps = _prep_inputs(x, w_qkv, b_qkv, w_proj)
    res = run_bass_kernel_spmd(nc, in_maps, core_ids=list(range(NCORE)), trace=trace)
    # host-side: sum row-parallel partials and add the folded biases
    # (attn @ (v + bv) = attn @ v + bv since softmax rows sum to 1)
    ybias = (b_qkv[2 * C :] @ w_proj + b_proj).astype(np.float32)
    out = np.empty((B, T, C), np.float32)
    for b in range(B):
        out[b] = (
            res.results[2 * b]["y"].astype(np.float32)
            + res.results[2 * b + 1]["y"].astype(np.float32)
            + ybias
        )
    return out, res


def kernel(x, w_qkv, b_qkv, w_proj, b_proj, n_heads=16):
    out, _ = _run(x, w_qkv, b_qkv, w_proj, b_proj, trace=False)
    return out
